# revision 11
# baseline (speedup 1.0000x reference)
"""GraphSAGE sim/cor dual-branch GNN on 8 Trainium2 NeuronCores.

Sharding: dst-node partition across 8 cores (per sharding hint). Host does
index preprocessing only (edge bucketing by dst shard, sort-by-dst, padding,
per-edge row gather = index lookup, 0/1 one-hot masks from indices,
count/reciprocal tables, dtype packing); all FP tensor arithmetic
(segment-mean via one-hot matmul on PE, FC layers, final mixing matmuls)
runs on device via Bass/Tile kernels.

Device-mapping choices (v2):
- per-edge feature rows are host-gathered (fp8e4, x64 scale) into a dense
  chunk-ordered stream [128, C, D] streamed by large contiguous HWDGE DMAs;
  the {0,1} one-hot stream [128, C, 32] is also host-built (pure index
  preprocessing) and streamed alongside - this removes the is_equal build
  from the DVE critical path entirely.
- aggregation matmuls are plain fp8 singles issued round-robin across the
  4 PE column groups (tile_position=(0,32s)); measured ~44 ns/chunk vs
  ~82 ns/chunk for DoubleRow pairs (LDWEIGHTS penalty dominates at FD=144).
- per-tile u tiles are persistent; the bias/mask column is pre-written once
  per launch. u transposes (PE transpose mode) land in one grouped PSUM
  bank; the PSUM->SBUF copies run on the Scalar (ACT) engine, keeping DVE
  for the recip-scale+self-add STT only.
- FC emission is deferred by one tile-group so the ACT copies overlap the
  next group's aggregation matmuls (no PE head-of-line stall).
- launch B streams layer-1 edge features in fp8 (power-of-2 scale folded
  into the recip table; self rows stay bf16), folds the b_out add into the
  ACT copy bias, and runs the z-mixing matmuls in bf16.

Math reformulation (linearity of mean-aggregation):
  layer0: u[d] = g[d] + mean_{e->d} g[src_e]; h0 = relu(u @ W_in + b_in*(1+[cnt>0]))
  layer1: out[d] = p[d] + mean_{e->d} p[src_e] + b_out, with p = h0 @ W_out
  z2sim = (1-a2-b2)*sim + (a2+b2(1-a1))*(cor@Wcs) + b2*a1*(sim@Wsc@Wcs)
  z2cor = (1-a2-b2)*cor + (a2+b2(1-a1))*(sim@Wsc) + b2*a1*(cor@Wcs@Wsc)
"""
import os
import numpy as np
import ml_dtypes

N0, N1, N2 = 200000, 50000, 10000
HID, OUT = 256, 128
DG = 144          # concat embedding dim
DGA = DG + 1      # augmented with bias/mask column
NC = 8
S1 = N1 // NC     # 6250 dst per core, layer0
S2 = N2 // NC     # 1250 dst per core, layer1
T1 = (S1 + 127) // 128   # 49 tiles
T2 = (S2 + 127) // 128   # 10 tiles
W = 32            # dst subtile width for one-hot aggregation
SUB = 128 // W    # subtiles per 128-dst tile
G = 4             # FC tiles per weight-load group (out 512 f32 = one PSUM bank)

_exec_times = []


def _pack_edges(src, dst, n_sub, ect):
    """Sort edges by dst, bucket into W-dst subtiles, pad subtile q to ect[q]
    chunks of 128 edge slots. Returns eidx [128, sum(ect)] int32 (src ids),
    dstl [128, sum(ect)] f32 (dst-local-in-subtile, -1 for pad)."""
    order = np.argsort(dst, kind="stable")
    src = src[order]
    dst = dst[order]
    tid = dst // W
    starts = np.concatenate([[0], np.cumsum(ect)])
    ctot = int(starts[-1])
    eidx = np.zeros((ctot * 128,), np.int32)
    dstl = np.full((ctot * 128,), -1.0, np.float32)
    bounds = np.searchsorted(tid, np.arange(n_sub + 1))
    for q in range(n_sub):
        a, b = bounds[q], bounds[q + 1]
        base = int(starts[q]) * 128
        n = b - a
        eidx[base:base + n] = src[a:b]
        dstl[base:base + n] = (dst[a:b] - q * W).astype(np.float32)
    eidx = eidx.reshape(ctot, 128).T.copy()
    dstl = dstl.reshape(ctot, 128).T.copy()
    return eidx, dstl


def _onehot(dstl):
    """{0,1} one-hot fp8 stream [128, C, W] from dst-local indices (index
    preprocessing; -1 pads give all-zero columns)."""
    f8 = ml_dtypes.float8_e4m3fn
    return (dstl[:, :, None] == np.arange(W, dtype=np.float32)[None, None, :]
            ).astype(f8)


def _balance_perm(cnt, n_sub, cap=512):
    """Assign dst slots to n_sub subtiles of (len(cnt)//n_sub) lanes each so
    per-subtile edge totals stay <= cap where feasible (minimal 128-slot
    chunk padding shared across SPMD shards); overflow steered to the last
    subtile. Returns perm: orig slot -> new position. Pure index shuffling."""
    nslots = cnt.shape[0]
    per = nslots // n_sub
    order = np.argsort(-cnt, kind="stable")
    assign = np.empty(nslots, np.int64)
    loads = np.zeros(n_sub, np.float64)
    tail = max(per - 4, 0)
    for r in range(tail):
        items = order[r * n_sub:(r + 1) * n_sub]
        rank = np.argsort(loads, kind="stable")
        assign[items] = rank
        loads[rank] += cnt[items]
    fill = np.full(n_sub, tail, np.int64)
    for i in order[tail * n_sub:]:
        c = cnt[i]
        open_ = fill < per
        fits = open_ & (loads + c <= cap)
        if fits.any():
            # best-fit: fullest bin that still fits under cap
            cand = np.where(fits)[0]
            b = cand[np.argmax(loads[cand])]
        else:
            # spill: highest-index open bin so overflow subtiles align
            # across SPMD shards (shared ect takes the max)
            b = int(np.where(open_)[0].max())
        assign[i] = b
        loads[b] += c
        fill[b] += 1
    idx_sorted = np.lexsort((np.arange(nslots), assign))
    perm = np.empty(nslots, np.int64)
    perm[idx_sorted] = np.arange(nslots)
    return perm


def _balance_perm2(ca, cb, n_sub, cap=512):
    """Joint two-branch balance: one permutation keeping both branches'
    per-subtile edge totals <= cap where feasible (overflow to last bin)."""
    nslots = ca.shape[0]
    per = nslots // n_sub
    order = np.argsort(-(ca + cb), kind="stable")
    la = np.zeros(n_sub)
    lb = np.zeros(n_sub)
    fill = np.zeros(n_sub, np.int64)
    assign = np.empty(nslots, np.int64)
    for i in order:
        open_ = fill < per
        fits = open_ & (la + ca[i] <= cap) & (lb + cb[i] <= cap)
        if fits.any():
            cand = np.where(fits)[0]
            b = cand[np.argmax(np.maximum(la, lb)[cand])]
        else:
            b = int(np.where(open_)[0].max())
        assign[i] = b
        la[b] += ca[i]
        lb[b] += cb[i]
        fill[b] += 1
    idx_sorted = np.lexsort((np.arange(nslots), assign))
    perm = np.empty(nslots, np.int64)
    perm[idx_sorted] = np.arange(nslots)
    return perm


def _shard_prep(e_src, e_dst, shard, n_sub, n_tiles):
    """Per-core edge lists (dst in shard) -> balanced-permuted locals +
    shared chunk counts + per-position degree tables."""
    nslots = n_tiles * 128
    lists = []
    ect = np.ones(n_sub, np.int64)
    for c in range(NC):
        m = (e_dst >= c * shard) & (e_dst < (c + 1) * shard)
        es, ed = e_src[m], e_dst[m] - c * shard
        cnt = np.bincount(ed, minlength=nslots).astype(np.int64)
        perm = _balance_perm(cnt, n_sub)
        ed = perm[ed]
        inv = np.empty(nslots, np.int64)
        inv[perm] = np.arange(nslots)
        cnt_new = cnt[inv].astype(np.float32)
        lists.append((es, ed, perm, inv, cnt_new))
        scnt = np.bincount(ed // W, minlength=n_sub)
        ect = np.maximum(ect, (scnt + 127) // 128)
    return lists, ect


def _recips(cnt, n_tiles):
    recip = 1.0 / np.maximum(cnt, 1.0)
    mask1p = 1.0 + (cnt > 0)
    return (recip.reshape(n_tiles, 128).T.copy(),
            mask1p.astype(np.float32).reshape(n_tiles, 128).T.copy())


def _build_launch_a(ect0):
    import concourse.bacc as bacc
    import concourse.mybir as mybir
    import concourse.tile as tile

    bf16 = mybir.dt.bfloat16
    f8 = mybir.dt.float8e4
    f32 = mybir.dt.float32
    nc = bacc.Bacc(enable_partition_id=False)
    C0 = int(ect0.sum())
    starts0 = np.concatenate([[0], np.cumsum(ect0)]).astype(int)
    # max chunks per FC-tile group (for medge/onehot tile sizing)
    ECMG = max(int(starts0[min(g0 + G, T1) * SUB] - starts0[g0 * SUB])
               for g0 in range(0, T1, G))
    g = {}
    for br in ("sim", "cor"):
        g[br] = dict(
            moh=nc.dram_tensor(f"moh_{br}", [128, C0, DG + W], f8, kind="ExternalInput"),
            recip=nc.dram_tensor(f"recip_{br}", [128, T1], f32, kind="ExternalInput"),
            mask1p=nc.dram_tensor(f"mask1p_{br}", [128, T1], f32, kind="ExternalInput"),
            win=nc.dram_tensor(f"win_{br}", [DGA, HID], bf16, kind="ExternalInput"),
            wout=nc.dram_tensor(f"wout_{br}", [HID, OUT], bf16, kind="ExternalInput"),
            gself=nc.dram_tensor(f"gself_{br}", [128, T1 * DG], bf16, kind="ExternalInput"),
            pt=nc.dram_tensor(f"pt_{br}", [128, T1 * 128], bf16, kind="ExternalOutput"),
        )
    ident_in = nc.dram_tensor("ident", [128, 128], bf16, kind="ExternalInput")

    with tile.TileContext(nc) as tc:
        with tc.tile_pool(name="const", bufs=1) as cp, \
             tc.tile_pool(name="medg", bufs=4) as mp_, \
             tc.tile_pool(name="work", bufs=3) as wp, \
             tc.tile_pool(name="grp", bufs=2) as hp, \
             tc.tile_pool(name="pagg", bufs=2, space="PSUM") as pagg, \
             tc.tile_pool(name="putg", bufs=2, space="PSUM") as putg, \
             tc.tile_pool(name="pfc", bufs=2, space="PSUM") as pfc, \
             tc.tile_pool(name="ppt", bufs=2, space="PSUM") as ppt:
            ident = cp.tile([128, 128], bf16)
            nc.sync.dma_start(out=ident[:], in_=ident_in[:])

            def emit_slab(br, g0, gn):
                # one combined edge-feature+one-hot slab DMA per group:
                # DMA_DIRECT2D issue cost (~0.6-1.1us each) on the sync queue
                # is what paces the stream, so fewer/larger is faster
                tt = g[br]
                gs0 = int(starts0[g0 * SUB])
                gs1 = int(starts0[(g0 + gn) * SUB])
                mo = mp_.tile([128, ECMG, DG + W], f8, tag="mo")
                nc.sync.dma_start(out=mo[:, :gs1 - gs0, :],
                                  in_=tt["moh"][:, gs0:gs1, :])
                return mo

            # first two groups' slabs go out before the remaining constants
            slab_q = {"sim": [], "cor": []}
            for g0 in (0, G):
                for br in ("sim", "cor"):
                    if g0 < T1:
                        slab_q[br].append(emit_slab(br, g0, min(G, T1 - g0)))

            cons = {}
            for br in ("sim", "cor"):
                tt = g[br]
                recip_t = cp.tile([128, T1], f32, tag=f"rc{br}")
                nc.sync.dma_start(out=recip_t[:], in_=tt["recip"][:])
                mask_t = cp.tile([128, T1], f32, tag=f"mk{br}")
                nc.sync.dma_start(out=mask_t[:], in_=tt["mask1p"][:])
                win_t = cp.tile([128, 2 * HID], bf16, tag=f"wi{br}")  # rows 0:128 | 128:145
                nc.sync.dma_start(out=win_t[:, :HID], in_=tt["win"][0:128, :])
                nc.sync.dma_start(out=win_t[:DGA - 128, HID:], in_=tt["win"][128:DGA, :])
                wout_t = cp.tile([128, 2 * OUT], bf16, tag=f"wo{br}")
                nc.sync.dma_start(out=wout_t[:, :OUT], in_=tt["wout"][0:128, :])
                nc.sync.dma_start(out=wout_t[:, OUT:], in_=tt["wout"][128:HID, :])
                gself_t = cp.tile([128, T1 * DG], bf16, tag=f"gs{br}")
                nc.sync.dma_start(out=gself_t[:], in_=tt["gself"][:])
                u_all = cp.tile([128, T1, DGA], bf16, tag=f"ua{br}")
                # pre-write the bias/mask column for every tile once
                nc.vector.tensor_copy(out=u_all[:, :, DG], in_=mask_t[:])
                cons[br] = (recip_t, mask_t, win_t, wout_t, gself_t, u_all)

            def emit_group(br, g0, gn, slab):
                tt = g[br]
                recip_t, mask_t, win_t, wout_t, gself_t, u_all = cons[br]
                gs0 = int(starts0[g0 * SUB])
                mo = slab
                # aggregation: round-robin across the 4 PE column groups
                for k in range(gn):
                    t = g0 + k
                    agg = pagg.tile([128, DG], f32, tag="agg")
                    qs = [(int(starts0[t * SUB + s]), int(starts0[t * SUB + s + 1]))
                          for s in range(SUB)]
                    maxj = max(b - a for a, b in qs)
                    for j in range(maxj):
                        for s in range(SUB):
                            a, b = qs[s]
                            if j >= b - a:
                                continue
                            c = a + j
                            nc.tensor.matmul(
                                agg[s * W:(s + 1) * W, :],
                                lhsT=mo[:, c - gs0, DG:],
                                rhs=mo[:, c - gs0, :DG],
                                start=(j == 0), stop=(j == b - a - 1),
                                tile_position=(0, s * W),
                                skip_group_check=True)
                    nc.vector.scalar_tensor_tensor(
                        out=u_all[:, t, :DG], in0=agg[:],
                        scalar=recip_t[:, t:t + 1],
                        in1=gself_t[:, t * DG:(t + 1) * DG],
                        op0=mybir.AluOpType.mult, op1=mybir.AluOpType.add)
                gw = gn * 128

                def fc_tail():
                    # transposes into one grouped PSUM bank (deferred so the
                    # PE never waits head-of-line on this group's STTs)
                    utg = putg.tile([128, 1024], bf16, tag="utg")
                    for k in range(gn):
                        t = g0 + k
                        nc.tensor.matmul(utg[:, k * 128:(k + 1) * 128],
                                         lhsT=u_all[:, t, :128], rhs=ident[:],
                                         is_transpose=True, skip_group_check=True)
                        nc.tensor.matmul(utg[:DGA - 128, 512 + k * 128:512 + (k + 1) * 128],
                                         lhsT=u_all[:, t, 128:DGA], rhs=ident[:],
                                         is_transpose=True, skip_group_check=True)
                    uta_g = hp.tile([128, G * 128], bf16, tag="uta")
                    nc.scalar.copy(out=uta_g[:, :gw], in_=utg[:, :gw])
                    utb_g = hp.tile([32, G * 128], bf16, tag="utb")
                    nc.scalar.copy(out=utb_g[:DGA - 128, :gw],
                                   in_=utg[:DGA - 128, 512:512 + gw])
                    h0_g = hp.tile([128, 2, G * 128], bf16, tag="h0")
                    for half in range(2):
                        fc = pfc.tile([128, G * 128], f32, tag="fc")
                        nc.tensor.matmul(fc[:, :gw],
                                         lhsT=win_t[:, half * 128:half * 128 + 128],
                                         rhs=uta_g[:, :gw], start=True, stop=False)
                        nc.tensor.matmul(fc[:, :gw],
                                         lhsT=win_t[:DGA - 128,
                                                    HID + half * 128:HID + half * 128 + 128],
                                         rhs=utb_g[:DGA - 128, :gw],
                                         start=False, stop=True)
                        nc.scalar.activation(out=h0_g[:, half, :gw], in_=fc[:, :gw],
                                             func=mybir.ActivationFunctionType.Relu)
                    pt_p = ppt.tile([128, G * 128], f32, tag="pt")
                    nc.tensor.matmul(pt_p[:, :gw], lhsT=wout_t[:, :OUT],
                                     rhs=h0_g[:, 0, :gw], start=True, stop=False)
                    nc.tensor.matmul(pt_p[:, :gw], lhsT=wout_t[:, OUT:],
                                     rhs=h0_g[:, 1, :gw], start=False, stop=True)
                    pt_s = wp.tile([128, G * 128], bf16, tag="pt_s")
                    nc.vector.tensor_copy(out=pt_s[:, :gw], in_=pt_p[:, :gw])
                    nc.gpsimd.dma_start(out=tt["pt"][:, g0 * 128:g0 * 128 + gw],
                                        in_=pt_s[:, :gw])
                return fc_tail

            pending = []
            for g0 in range(0, T1, G):
                gn = min(G, T1 - g0)
                for br in ("sim", "cor"):
                    slab = (slab_q[br].pop(0) if slab_q[br]
                            else emit_slab(br, g0, gn))
                    tail = emit_group(br, g0, gn, slab)
                    pending.append(tail)
                    # deferred by one group: FC runs while the next group's
                    # aggregation keeps the PE busy past the ACT copies
                    if len(pending) > 2:
                        pending.pop(0)()
            for tail in pending:
                tail()
    nc.compile()
    return nc


def _build_launch_b(ect1, coef):
    import concourse.bacc as bacc
    import concourse.mybir as mybir
    import concourse.tile as tile

    bf16 = mybir.dt.bfloat16
    f8 = mybir.dt.float8e4
    f32 = mybir.dt.float32
    nc = bacc.Bacc(enable_partition_id=False)
    C1 = int(ect1.sum())
    starts1 = np.concatenate([[0], np.cumsum(ect1)]).astype(int)
    GB = 4
    ECMT = max(int(starts1[min(g0 + GB, T2) * SUB] - starts1[g0 * SUB])
               for g0 in range(0, T2, GB))
    a1, a2, b2 = coef
    cbase = float(1 - a2 - b2)
    c1 = float(a2 + b2 * (1 - a1))
    c2 = float(b2 * a1)
    g = {}
    for br in ("sim", "cor"):
        g[br] = dict(
            moh=nc.dram_tensor(f"moh_{br}", [128, C1, OUT + W], f8, kind="ExternalInput"),
            recip=nc.dram_tensor(f"recip_{br}", [128, T2], f32, kind="ExternalInput"),
            ptselfd=nc.dram_tensor(f"ptselfd_{br}", [128, T2 * OUT], bf16, kind="ExternalInput"),
            zt=nc.dram_tensor(f"zt_{br}", [128, T2 * 128], f32, kind="ExternalOutput"),
        )
    wcs_in = nc.dram_tensor("wcs", [OUT, OUT], f32, kind="ExternalInput")
    wsc_in = nc.dram_tensor("wsc", [OUT, OUT], f32, kind="ExternalInput")
    bo_in = nc.dram_tensor("bo", [128, 2], f32, kind="ExternalInput")
    identf_in = nc.dram_tensor("identf", [128, 128], f32, kind="ExternalInput")
    ident_in = nc.dram_tensor("ident", [128, 128], bf16, kind="ExternalInput")

    with tile.TileContext(nc) as tc:
        with tc.tile_pool(name="const", bufs=1) as cp, \
             tc.tile_pool(name="medg", bufs=4) as mp_, \
             tc.tile_pool(name="work", bufs=4) as wp, \
             tc.tile_pool(name="pagg", bufs=2, space="PSUM") as pagg, \
             tc.tile_pool(name="putr", bufs=2, space="PSUM") as putr, \
             tc.tile_pool(name="pmix", bufs=2, space="PSUM") as pmix:
            def emit_slab(br, g0, gn):
                tt = g[br]
                gs0 = int(starts1[g0 * SUB])
                gs1 = int(starts1[(g0 + gn) * SUB])
                mo = mp_.tile([128, ECMT, OUT + W], f8, tag="mo")
                nc.sync.dma_start(out=mo[:, :gs1 - gs0, :],
                                  in_=tt["moh"][:, gs0:gs1, :])
                return mo

            slab_q = {"sim": [], "cor": []}
            for g0 in (0, GB):
                for br in ("sim", "cor"):
                    if g0 < T2:
                        slab_q[br].append(emit_slab(br, g0, min(GB, T2 - g0)))

            identf = cp.tile([128, 128], f32)
            nc.sync.dma_start(out=identf[:], in_=identf_in[:])
            ident = cp.tile([128, 128], bf16)
            nc.sync.dma_start(out=ident[:], in_=ident_in[:])
            wcs = cp.tile([128, OUT], f32)
            nc.sync.dma_start(out=wcs[:], in_=wcs_in[:])
            wsc = cp.tile([128, OUT], f32)
            nc.sync.dma_start(out=wsc[:], in_=wsc_in[:])
            bo = cp.tile([128, 2], f32)
            nc.sync.dma_start(out=bo[:], in_=bo_in[:])
            # one-time: pre-scaled stationary matrices for the z-mixing
            # P = Wsc@Wcs (for sim), Q = Wcs@Wsc (for cor); bf16 stationaries
            wcsT_p = pmix.tile([128, 128], f32, tag="z")
            nc.tensor.transpose(out=wcsT_p[:], in_=wcs[:], identity=identf[:])
            wcsT = cp.tile([128, 128], f32, tag="wcsT")
            nc.vector.tensor_copy(out=wcsT[:], in_=wcsT_p[:])
            wscT_p = pmix.tile([128, 128], f32, tag="z")
            nc.tensor.transpose(out=wscT_p[:], in_=wsc[:], identity=identf[:])
            wscT = cp.tile([128, 128], f32, tag="wscT")
            nc.vector.tensor_copy(out=wscT[:], in_=wscT_p[:])
            P_p = pmix.tile([128, 128], f32, tag="z")
            nc.tensor.matmul(P_p[:], lhsT=wscT[:], rhs=wcs[:], start=True, stop=True)
            P_c2 = cp.tile([128, 128], bf16, tag="P")
            nc.vector.tensor_scalar_mul(P_c2[:], P_p[:], c2)
            Q_p = pmix.tile([128, 128], f32, tag="z")
            nc.tensor.matmul(Q_p[:], lhsT=wcsT[:], rhs=wsc[:], start=True, stop=True)
            Q_c2 = cp.tile([128, 128], bf16, tag="Q")
            nc.vector.tensor_scalar_mul(Q_c2[:], Q_p[:], c2)
            wcs_c1 = cp.tile([128, 128], bf16, tag="wcs1")
            nc.vector.tensor_scalar_mul(wcs_c1[:], wcs[:], c1)
            wsc_c1 = cp.tile([128, 128], bf16, tag="wsc1")
            nc.vector.tensor_scalar_mul(wsc_c1[:], wsc[:], c1)

            tiles = {}
            for br in ("sim", "cor"):
                tt = g[br]
                recip_t = cp.tile([128, T2], f32, tag=f"rc{br}")
                nc.sync.dma_start(out=recip_t[:], in_=tt["recip"][:])
                ptself_t = cp.tile([128, T2 * OUT], bf16, tag=f"ps{br}")
                nc.sync.dma_start(out=ptself_t[:], in_=tt["ptselfd"][:])
                tiles[br] = (recip_t, ptself_t)

            def emit_bgroup(g0, gn):
                gw = gn * 128
                gs0 = int(starts1[g0 * SUB])
                u1s = {}
                for bi, br in enumerate(("sim", "cor")):
                    recip_t, ptself_t = tiles[br]
                    mo = (slab_q[br].pop(0) if slab_q[br]
                          else emit_slab(br, g0, gn))
                    u1s[br] = []
                    for k in range(gn):
                        t = g0 + k
                        agg = pagg.tile([128, OUT], f32, tag="agg")
                        qs = [(int(starts1[t * SUB + s]), int(starts1[t * SUB + s + 1]))
                              for s in range(SUB)]
                        maxj = max(b - a for a, b in qs)
                        for j in range(maxj):
                            for s in range(SUB):
                                a, b = qs[s]
                                if j >= b - a:
                                    continue
                                c = a + j
                                nc.tensor.matmul(
                                    agg[s * W:(s + 1) * W, :],
                                    lhsT=mo[:, c - gs0, OUT:],
                                    rhs=mo[:, c - gs0, :OUT],
                                    start=(j == 0), stop=(j == b - a - 1),
                                    tile_position=(0, s * W),
                                    skip_group_check=True)
                        u1 = wp.tile([128, OUT], bf16, tag=f"u1{br}{k}")
                        nc.vector.scalar_tensor_tensor(
                            out=u1[:], in0=agg[:], scalar=recip_t[:, t:t + 1],
                            in1=ptself_t[:, t * OUT:(t + 1) * OUT],
                            op0=mybir.AluOpType.mult, op1=mybir.AluOpType.add)
                        u1s[br].append(u1)

                def tail():
                    sg = {}
                    for bi, br in enumerate(("sim", "cor")):
                        utg = putr.tile([128, 512], bf16, tag="utg")
                        for k, u1 in enumerate(u1s[br]):
                            nc.tensor.matmul(utg[:, k * 128:(k + 1) * 128],
                                             lhsT=u1[:], rhs=ident[:],
                                             is_transpose=True,
                                             skip_group_check=True)
                        sgt = wp.tile([128, GB * 128], bf16, tag=f"sg{br}")
                        # PSUM->SBUF copy with b_out folded into the bias
                        nc.scalar.activation(
                            out=sgt[:, :gw], in_=utg[:, :gw],
                            func=mybir.ActivationFunctionType.Identity,
                            bias=bo[:, bi:bi + 1])
                        sg[br] = sgt
                    sTg, cTg = sg["sim"], sg["cor"]
                    for br, base, other, w_c1, m_c2 in (
                            ("sim", sTg, cTg, wcs_c1, P_c2),
                            ("cor", cTg, sTg, wsc_c1, Q_c2)):
                        z_p = pmix.tile([128, GB * 128], f32, tag="z")
                        nc.tensor.matmul(z_p[:, :gw], lhsT=w_c1[:],
                                         rhs=other[:, :gw],
                                         start=True, stop=False)
                        nc.tensor.matmul(z_p[:, :gw], lhsT=m_c2[:],
                                         rhs=base[:, :gw],
                                         start=False, stop=True)
                        z = wp.tile([128, GB * 128], f32, tag=f"z{br}")
                        nc.vector.scalar_tensor_tensor(
                            out=z[:, :gw], in0=base[:, :gw], scalar=cbase,
                            in1=z_p[:, :gw],
                            op0=mybir.AluOpType.mult, op1=mybir.AluOpType.add)
                        nc.gpsimd.dma_start(
                            out=g[br]["zt"][:, g0 * 128:g0 * 128 + gw],
                            in_=z[:, :gw])
                return tail

            pending = None
            for g0 in range(0, T2, GB):
                gn = min(GB, T2 - g0)
                tail = emit_bgroup(g0, gn)
                if pending is not None:
                    pending()
                pending = tail
            pending()
    nc.compile()
    return nc


FP8_SCALE = 64.0  # power-of-2 scale into fp8e4m3 normal range


def _prep_a(inputs):
    bf16 = ml_dtypes.bfloat16
    f8 = ml_dtypes.float8_e4m3fn
    x = np.asarray(inputs["x"]).astype(np.int64)
    branches = {}
    for br in ("sim", "cor"):
        tabs = [np.asarray(inputs[f"emb_{br}_{i}"], np.float32) for i in range(5)]
        gtab = np.concatenate([tabs[i][x[:, i]] for i in range(5)], axis=1)
        win = np.asarray(inputs[f"W_in_{br}"], np.float32)
        bin_ = np.asarray(inputs[f"b_in_{br}"], np.float32)
        win_aug = np.concatenate([win, bin_[None, :]], 0)
        branches[br] = dict(
            gtab8=np.ascontiguousarray(
                np.clip(gtab * FP8_SCALE, -448, 448).astype(f8)),
            gtab=np.ascontiguousarray(gtab.astype(bf16)),
            win=win_aug.astype(bf16),
            wout=np.asarray(inputs[f"W_out_{br}"], np.float32).astype(bf16),
            bout=np.asarray(inputs[f"b_out_{br}"], np.float32),
            e0s=np.asarray(inputs[f"e0_{br}_src"]).astype(np.int64),
            e0d=np.asarray(inputs[f"e0_{br}_dst"]).astype(np.int64),
            e1s=np.asarray(inputs[f"e1_{br}_src"]).astype(np.int64),
            e1d=np.asarray(inputs[f"e1_{br}_dst"]).astype(np.int64),
        )

    ect0 = np.ones(T1 * SUB, np.int64)
    shardinfo = {}
    for br in ("sim", "cor"):
        lists, ect = _shard_prep(branches[br]["e0s"], branches[br]["e0d"], S1,
                                 T1 * SUB, T1)
        shardinfo[br] = lists
        ect0 = np.maximum(ect0, ect)
    ident = np.eye(128, dtype=bf16)
    perms = {}
    in_maps = []
    for c in range(NC):
        im = {"ident": ident}
        for br in ("sim", "cor"):
            bb = branches[br]
            es, ed, perm, inv, cnt_new = shardinfo[br][c]
            perms[(br, c)] = perm
            eidx, dstl = _pack_edges(es, ed, T1 * SUB, ect0)
            recip, mask1p = _recips(cnt_new, T1)
            gself = np.zeros((T1 * 128, DG), bf16)
            lo = c * S1
            hi = min(lo + T1 * 128, N0)
            gself[:hi - lo] = bb["gtab"][lo:hi]
            gself = gself[inv]                             # permuted positions
            moh = np.concatenate([bb["gtab8"][eidx], _onehot(dstl)], axis=2)
            im.update({
                f"moh_{br}": np.ascontiguousarray(moh),    # [128, C0, DG+W]
                f"recip_{br}": recip / FP8_SCALE, f"mask1p_{br}": mask1p,
                f"win_{br}": bb["win"], f"wout_{br}": bb["wout"],
                f"gself_{br}": np.ascontiguousarray(
                    gself.reshape(T1, 128, DG).transpose(1, 0, 2).reshape(128, T1 * DG)),
            })
        in_maps.append(im)
    return ect0, in_maps, branches, perms


def _prep_b(inputs, branches, ptabs):
    bf16 = ml_dtypes.bfloat16
    f8 = ml_dtypes.float8_e4m3fn
    a1 = float(np.asarray(inputs["a1"]).ravel()[0])
    a2 = float(np.asarray(inputs["a2"]).ravel()[0])
    b2 = float(np.asarray(inputs["b2"]).ravel()[0])
    # one shared permutation per core (z-mixing pairs sim/cor at same dst)
    nslots = T2 * 128
    ect1 = np.ones(T2 * SUB, np.int64)
    shard_b = []
    for c in range(NC):
        data = {}
        cnts = {}
        for br in ("sim", "cor"):
            e_src = branches[br]["e1s"]
            e_dst = branches[br]["e1d"]
            m = (e_dst >= c * S2) & (e_dst < (c + 1) * S2)
            es, ed = e_src[m], e_dst[m] - c * S2
            data[br] = (es, ed)
            cnts[br] = np.bincount(ed, minlength=nslots).astype(np.int64)
        perm = _balance_perm2(cnts["sim"], cnts["cor"], T2 * SUB)
        inv = np.empty(nslots, np.int64)
        inv[perm] = np.arange(nslots)
        ent = {"perm": perm, "inv": inv}
        for br in ("sim", "cor"):
            es, ed = data[br]
            ed = perm[ed]
            ent[br] = (es, ed, cnts[br][inv].astype(np.float32))
            scnt = np.bincount(ed // W, minlength=T2 * SUB)
            ect1 = np.maximum(ect1, (scnt + 127) // 128)
        shard_b.append(ent)
    identf = np.eye(128, dtype=np.float32)
    bo = np.zeros((128, 2), np.float32)
    bo[:, 0] = branches["sim"]["bout"]
    bo[:, 1] = branches["cor"]["bout"]
    # fp8 pack of the p tables with a power-of-2 scale folded into recip
    pmax = max(float(np.abs(ptabs[br]).max()) for br in ("sim", "cor"))
    pscale = float(2.0 ** np.floor(np.log2(448.0 / max(pmax, 1e-30))))
    pscale = min(max(pscale, 1.0), 512.0)
    ptab_bfs = {br: ptabs[br].astype(bf16) for br in ("sim", "cor")}
    ptab_f8s = {br: np.clip(ptabs[br] * pscale, -448, 448).astype(f8)
                for br in ("sim", "cor")}
    perms1 = {}
    in_maps = []
    for c in range(NC):
        im = {"bo": bo, "identf": identf,
              "ident": np.eye(128, dtype=bf16),
              "wcs": np.asarray(inputs["W_cor2sim"], np.float32),
              "wsc": np.asarray(inputs["W_sim2cor"], np.float32)}
        ent = shard_b[c]
        perm, inv = ent["perm"], ent["inv"]
        perms1[c] = perm
        for br in ("sim", "cor"):
            es, ed, cnt_new = ent[br]
            eidx, dstl = _pack_edges(es, ed, T2 * SUB, ect1)
            recip, _ = _recips(cnt_new, T2)
            ptself = np.zeros((T2 * 128, OUT), bf16)
            lo = c * S2
            hi = min(lo + T2 * 128, N1)
            ptself[:hi - lo] = ptab_bfs[br][lo:hi]
            ptself = ptself[inv]                           # permuted positions
            moh = np.concatenate([ptab_f8s[br][eidx], _onehot(dstl)], axis=2)
            im.update({
                f"moh_{br}": np.ascontiguousarray(moh),    # [128, C1, OUT+W]
                f"recip_{br}": recip / pscale,
                f"ptselfd_{br}": np.ascontiguousarray(
                    ptself.reshape(T2, 128, OUT).transpose(1, 0, 2).reshape(128, T2 * OUT)),
            })
        in_maps.append(im)
    return ect1, in_maps, (a1, a2, b2), perms1


def kernel(**inputs):
    from concourse.bass_utils import run_bass_kernel_spmd
    global _exec_times
    _exec_times = []
    trace = os.environ.get("BASS_KERNEL_TRACE", "0") == "1"
    tkw = {}
    if trace:
        import sys, types
        import antenv
        from trn_agent_boot.trn_boot import _ntff_profile_via_ctypes
        if "antenv.axon_hooks" not in sys.modules:
            mod = types.ModuleType("antenv.axon_hooks")
            mod.get_axon_ntff_profile_hook = (
                lambda: _ntff_profile_via_ctypes("/opt/axon/libaxon_pjrt.so"))
            mod.set_axon_ntff_profile_hook = lambda h: None
            sys.modules["antenv.axon_hooks"] = mod
            antenv.axon_hooks = mod

    ect0, in_maps, branches, perms = _prep_a(inputs)
    nc_a = _build_launch_a(ect0)
    if trace:
        import shutil
        shutil.rmtree("/root/problem/work/trace_A", ignore_errors=True)
        os.makedirs("/root/problem/work/trace_A", exist_ok=True)
        tkw = {"tmpdir": "/root/problem/work/trace_A"}
    res_a = run_bass_kernel_spmd(nc_a, in_maps, core_ids=list(range(NC)),
                                 trace=trace, **tkw)
    if trace:
        _exec_times.append(res_a.exec_time_ns)

    # ---- host exchange: assemble p [N1, OUT] per branch ----
    ptabs = {}
    for br in ("sim", "cor"):
        cols = []
        for c in range(NC):
            pt = np.asarray(res_a.results[c][f"pt_{br}"])  # [128, T1*128] bf16
            cols.append(pt.T[perms[(br, c)][:S1]])         # unpermute dst rows
        ptabs[br] = np.ascontiguousarray(
            np.concatenate(cols, 0)).astype(np.float32)

    ect1, in_maps, coef, perms1 = _prep_b(inputs, branches, ptabs)
    nc_b = _build_launch_b(ect1, coef)
    if trace:
        import shutil
        shutil.rmtree("/root/problem/work/trace_B", ignore_errors=True)
        os.makedirs("/root/problem/work/trace_B", exist_ok=True)
        tkw = {"tmpdir": "/root/problem/work/trace_B"}
    res_b = run_bass_kernel_spmd(nc_b, in_maps, core_ids=list(range(NC)),
                                 trace=trace, **tkw)
    if trace:
        _exec_times.append(res_b.exec_time_ns)

    outs = {}
    for br in ("sim", "cor"):
        rows = []
        for c in range(NC):
            zt = np.asarray(res_b.results[c][f"zt_{br}"])   # [128, T2*128]
            rows.append(zt.T[perms1[c][:S2]])               # unpermute dst rows
        outs[br] = np.ascontiguousarray(np.concatenate(rows, 0), dtype=np.float32)
    return outs["sim"], outs["cor"]
